# revision 16
# baseline (speedup 1.0000x reference)
"""Causal self-attention kernel for 8 TRN2 NeuronCores.

Problem: B=4, T=2048, C=1024, H=16 heads, D=64 (fp32 in/out).

Sharding: 8 cores = 4 batch entries x 2 head-groups (8 heads each).
Each core computes, for its (batch b, head-group hg):
    qkv slice -> flash-style causal attention (no-max softmax) -> partial
    projection y_part = attn_out @ W_proj[rows of its heads].
Host sums the two partial projections per batch entry.

Key optimizations over the 262us baseline:
  - qkv projections run as fp8e4m3 DoubleRow matmuls (0.5 cyc/row, 2 k-tiles
    per instruction => 2.7x bf16 MAC throughput).  Accuracy is preserved with
    a 3-term compensated product:
        x@W ~= x8@w8 + xr@w8 + (x8/64)@(wr*64)
    where x8=fp8(x), xr=fp8(x-x8), w8=fp8(W), wr=W-w8.  Measured error is
    BETTER than bf16 (residuals capture the quantization error; only the
    xr@wr cross term ~0.07% is dropped).  All splits are precomputed on host.
  - att@V is restructured: stationary = P^T chunk [128k x 128q], moving =
    [v_h | ones] (65 cols), output psY[128 queries, 65] -- full 128 output
    partitions instead of 65, halving PE time vs the baseline layout.  Row 64
    of psY is the softmax denominator for free.
  - normalization is a per-partition broadcast multiply (recip of den column),
    no GPSIMD partition_broadcast needed.
  - the [q, d] -> [d, q] transpose for the projection runs on the DMA XBAR
    (dma_start_transpose), costing no PE/DVE time.
  - PSUM: one start/stop per psum BANK per accumulation lifetime (the sim
    zeroes/tracks groups at 2KB granularity); the 2x2x65 psY accumulator
    regions inside one bank rely on deferred first-touch zeroing.

Cost-model (TimelineSim) breakdown per core: PE ~176us busy, ACT ~146us
(exp), DVE ~77us (copies, tri mask, normalize), DMA ~60us.
"""

import numpy as np
import ml_dtypes
import sys

sys.path.insert(0, "/opt/trn_rl_repo")

import concourse.bass as bass
import concourse.mybir as mybir
import concourse.tile as tile
from concourse import bacc
from concourse.bass_utils import run_bass_kernel_spmd

BF = mybir.dt.bfloat16
F8 = mybir.dt.float8e4
F32 = mybir.dt.float32
AF = mybir.ActivationFunctionType
DR = mybir.MatmulPerfMode.DoubleRow

B, T, C = 4, 2048, 1024
H, D = 16, 64
N_CORES = 8
HEADS_PER_CORE = 8          # 4 pairs
PAIRS = 4
TC = T // 128               # 16 t-chunks of 128
TG = T // 512               # 4 t-groups of 512
CT = C // 128               # 8 contraction tiles of 128

_compiled = None


def _build():
    nc = bacc.Bacc("TRN2", target_bir_lowering=False)

    x8 = nc.declare_dram_parameter("x8", [C, T], F8, isOutput=False)
    xr = nc.declare_dram_parameter("xr", [C, T], F8, isOutput=False)
    x8s = nc.declare_dram_parameter("x8s", [C, T], F8, isOutput=False)
    w8q = nc.declare_dram_parameter("w8q", [C, 512], F8, isOutput=False)
    w8k = nc.declare_dram_parameter("w8k", [C, 512], F8, isOutput=False)
    w8v = nc.declare_dram_parameter("w8v", [C, 512], F8, isOutput=False)
    wrq = nc.declare_dram_parameter("wrq", [C, 512], F8, isOutput=False)
    wrk = nc.declare_dram_parameter("wrk", [C, 512], F8, isOutput=False)
    wrv = nc.declare_dram_parameter("wrv", [C, 512], F8, isOutput=False)
    wp = nc.declare_dram_parameter("wp", [512, C], BF, isOutput=False)
    tri = nc.declare_dram_parameter("tri", [128, 128], BF, isOutput=False)
    y = nc.declare_dram_parameter("y", [T, C], F32, isOutput=True)

    with tile.TileContext(nc) as tc:
        with (
            tc.tile_pool(name="const", bufs=1) as cpool,
            tc.tile_pool(name="small", bufs=2) as spool,
            tc.tile_pool(name="ps_s", bufs=2, space="PSUM") as ps_s,
            tc.tile_pool(name="ps_sm", bufs=4, space="PSUM") as ps_sm,
        ):
            # ---------------- SBUF tiles ----------------
            x8_t = cpool.tile([128, CT, T], F8, name="x8_t", tag="x8_t")
            xr_t = cpool.tile([128, CT, T], F8, name="xr_t", tag="xr_t")
            x8s_t = cpool.tile([128, CT, T], F8, name="x8s_t", tag="x8s_t")
            w8_t = {}
            wr_t = {}
            for nm, w8d, wrd in (("q", w8q, wrq), ("k", w8k, wrk), ("v", w8v, wrv)):
                w8_t[nm] = cpool.tile([128, CT, 512], F8, name=f"w8{nm}_t",
                                      tag=f"w8{nm}_t")
                wr_t[nm] = cpool.tile([128, CT, 512], F8, name=f"wr{nm}_t",
                                      tag=f"wr{nm}_t")
            wp_t = cpool.tile([128, PAIRS, C], BF, name="wp_t", tag="wp_t")
            tri_t = cpool.tile([128, 128], BF, name="tri_t", tag="tri_t")

            # v tiles: [t-chunk, head, 64 v dims | ones | pad]
            v_t = cpool.tile([128, TC, HEADS_PER_CORE, 66], BF, name="v_t",
                             tag="v_t")
            nc.vector.memset(v_t[:, :, :, 64:65], 1.0)

            q_t = [[cpool.tile([128, 512], BF, name=f"q_{p}_{g}", tag=f"q_{p}_{g}")
                    for g in range(TG)] for p in range(PAIRS)]
            k_t = [[cpool.tile([128, 512], BF, name=f"k_{p}_{g}", tag=f"k_{p}_{g}")
                    for g in range(TG)] for p in range(PAIRS)]
            # transposed attention output, one [128 = 2h x 64d, 128 t] tile per
            # (pair, group, qtile): the XBAR transpose needs an offset-0,
            # whole-tile destination (nonzero free offsets corrupt the output)
            yT2_t = {(p, g, qt): cpool.tile([128, 128], BF,
                                            name=f"yT2_{p}_{g}_{qt}",
                                            tag=f"yT2_{p}_{g}_{qt}")
                     for p in range(PAIRS) for g in range(TG) for qt in range(4)}

            # ---------------- DMA loads ----------------
            x8_r = x8.ap().rearrange("(ct p) t -> p ct t", p=128)
            xr_r = xr.ap().rearrange("(ct p) t -> p ct t", p=128)
            x8s_r = x8s.ap().rearrange("(ct p) t -> p ct t", p=128)
            w8_r = {nm: d.ap().rearrange("(ct p) d -> p ct d", p=128)
                    for nm, d in (("q", w8q), ("k", w8k), ("v", w8v))}
            wr_r = {nm: d.ap().rearrange("(ct p) d -> p ct d", p=128)
                    for nm, d in (("q", wrq), ("k", wrk), ("v", wrv))}

            def load_xg(g):
                tg = slice(g * 512, (g + 1) * 512)
                nc.sync.dma_start(x8_t[:, :, tg], x8_r[:, :, tg])
                nc.sync.dma_start(xr_t[:, :, tg], xr_r[:, :, tg])
                nc.sync.dma_start(x8s_t[:, :, tg], x8s_r[:, :, tg])

            # startup-critical order: the first matmul (k of pair0/group0,
            # term x8@w8k) needs only x8 g0 + w8k
            nc.sync.dma_start(x8_t[:, 0:4, 0:512], x8_r[:, 0:4, 0:512])
            nc.sync.dma_start(w8_t["k"][:, 0:4], w8_r["k"][:, 0:4])
            nc.sync.dma_start(x8_t[:, 4:8, 0:512], x8_r[:, 4:8, 0:512])
            nc.sync.dma_start(w8_t["k"][:, 4:8], w8_r["k"][:, 4:8])
            nc.sync.dma_start(w8_t["q"][:], w8_r["q"][:])
            nc.sync.dma_start(xr_t[:, :, 0:512], xr_r[:, :, 0:512])
            nc.sync.dma_start(wr_t["k"][:], wr_r["k"][:])
            nc.sync.dma_start(wr_t["q"][:], wr_r["q"][:])
            nc.sync.dma_start(x8s_t[:, :, 0:512], x8s_r[:, :, 0:512])
            nc.sync.dma_start(tri_t[:], tri.ap())
            nc.sync.dma_start(w8_t["v"][:], w8_r["v"][:])
            nc.sync.dma_start(wr_t["v"][:], wr_r["v"][:])
            for g in range(1, TG):
                load_xg(g)
            nc.sync.dma_start(wp_t[:], wp.ap().rearrange("(pr p) co -> p pr co",
                                                         p=128))

            # ---------------- qkv work items (3-term fp8 DoubleRow) ----------
            def emit_v_chunk(tc16):
                psV = ps_sm.tile([128, 512], F32, name=f"psV_{tc16}", tag="util",
                                 bufs=2)
                tsl = slice(tc16 * 128, (tc16 + 1) * 128)
                terms = [(x8_t, w8_t["v"]), (xr_t, w8_t["v"]), (x8s_t, wr_t["v"])]
                for ti, (xa, wb) in enumerate(terms):
                    for ci in range(4):
                        nc.tensor.matmul(
                            psV[:],
                            xa[:, 2 * ci:2 * ci + 2, tsl],
                            wb[:, 2 * ci:2 * ci + 2, :],
                            start=(ti == 0 and ci == 0),
                            stop=(ti == 2 and ci == 3),
                            perf_mode=DR,
                        )
                nc.vector.tensor_copy(v_t[:, tc16, :, 0:64], psV[:])

            def emit_qk_group(p, g, which):
                dest = q_t if which == "q" else k_t
                ps = ps_sm.tile([128, 512], F32, name=f"ps{which}_{p}_{g}",
                                tag="util", bufs=2)
                psl = slice(p * 128, (p + 1) * 128)
                gsl = slice(g * 512, (g + 1) * 512)
                terms = [(w8_t[which], x8_t), (w8_t[which], xr_t),
                         (wr_t[which], x8s_t)]
                for ti, (wa, xb) in enumerate(terms):
                    for ci in range(4):
                        nc.tensor.matmul(
                            ps[:],
                            wa[:, 2 * ci:2 * ci + 2, psl],
                            xb[:, 2 * ci:2 * ci + 2, gsl],
                            start=(ti == 0 and ci == 0),
                            stop=(ti == 2 and ci == 3),
                            perf_mode=DR,
                        )
                nc.vector.tensor_copy(dest[p][g][:], ps[:])

            def all_qkv_items(p):
                items = []
                for g in range(TG):
                    items.append(lambda p=p, g=g: emit_qk_group(p, g, "k"))
                    items.append(lambda p=p, g=g: emit_qk_group(p, g, "q"))
                return items

            # ---------------- attention for one (pair, group) ----------------
            def emit_attention_group(p, g, fillers, tail_proj=False):
                nchunks = 4 * g + 4
                # two accumulator banks: lo = qtiles 0,1 / hi = qtiles 2,3
                # layout [128 q, qt%2, h, 65]; col 64 = denominator
                psY = [ps_sm.tile([128, 2, 2, 65], F32, name=f"psY{half}_{p}_{g}",
                                  tag="psY", bufs=2) for half in range(2)]

                def finalize_qt(qt):
                    ps = psY[qt // 2]
                    rec = spool.tile([128, 2, 1], F32, name="rec", tag="rec",
                                     bufs=4)
                    nc.vector.reciprocal(rec[:], ps[:, qt % 2, :, 64:65])
                    ysb = spool.tile([128, 2, 64], BF, name="ysb", tag="ysb",
                                     bufs=4)
                    nc.vector.tensor_mul(
                        ysb[:], ps[:, qt % 2, :, 0:64],
                        rec[:].to_broadcast([128, 2, 64]),
                    )
                    # [q, (h d)] -> [(h d), q] on the DMA crossbar
                    nc.sync.dma_start_transpose(yT2_t[(p, g, qt)][:], ysb[:])

                started = [False, False]

                def y_mm(pT, c, qt, h):
                    half = qt // 2
                    st = not started[half]
                    started[half] = True
                    # one start/stop per psum BANK: start on the first emitted
                    # matmul into the tile, stop on the very last (the sim
                    # zeroes/tracks accumulation groups per 2KB region)
                    stop = qt % 2 == 1 and h == 1 and c == 4 * g + qt
                    nc.tensor.matmul(
                        psY[half][:, qt % 2, h, 0:65],
                        pT[:, h, qt * 128:(qt + 1) * 128],
                        v_t[:, c, 2 * p + h, 0:65],
                        start=st, stop=stop,
                    )

                for c in range(nchunks):
                    diag = c >= 4 * g
                    jofs = 128 * (c - 4 * g) if diag else 0
                    kg, kc = c // 4, c % 4
                    psS = ps_s.tile([128, 2, 512], F32, name=f"psS_{p}_{g}_{c}",
                                    tag="s")
                    for h in range(2):
                        nc.tensor.matmul(
                            psS[:, h, jofs:512],
                            k_t[p][kg][h * 64:(h + 1) * 64,
                                       kc * 128:(kc + 1) * 128],
                            q_t[p][g][h * 64:(h + 1) * 64, jofs:512],
                            start=True, stop=True,
                        )
                    pT = spool.tile([128, 2, 512], BF, name="pT", tag="pT",
                                    bufs=6)
                    nc.scalar.activation(pT[:, :, jofs:512], psS[:, :, jofs:512],
                                         AF.Exp, scale=0.125)
                    qt_min = c - 4 * g if diag else 0
                    # non-diagonal qtiles first (they don't need the tri mask)
                    for qt in range(qt_min + 1, 4):
                        for h in range(2):
                            y_mm(pT, c, qt, h)
                    if diag:
                        nc.vector.tensor_mul(
                            pT[:, :, jofs:jofs + 128],
                            pT[:, :, jofs:jofs + 128],
                            tri_t[:, None, :].to_broadcast([128, 2, 128]),
                        )
                    for h in range(2):
                        y_mm(pT, c, qt_min, h)
                    if diag:
                        finalize_qt(qt_min)
                        if tail_proj and qt_min >= 1:
                            # last group: its proj chunks chase the transposes
                            emit_proj_chunk(4 * g + qt_min - 1)
                    if fillers and c % 2 == 1:
                        fillers.pop(0)()
                if tail_proj:
                    emit_proj_chunk(4 * g + 3)

            # ---------------- projection chunk ----------------
            def emit_proj_chunk(tc16):
                g16, qt16 = tc16 // 4, tc16 % 4
                for co2 in range(2):
                    psZ = ps_sm.tile([128, 512], F32, name=f"psZ_{tc16}_{co2}",
                                     tag="util", bufs=2)
                    for p in range(PAIRS):
                        nc.tensor.matmul(
                            psZ[:],
                            yT2_t[(p, g16, qt16)][:],
                            wp_t[:, p, co2 * 512:(co2 + 1) * 512],
                            start=(p == 0), stop=(p == PAIRS - 1),
                        )
                    z = spool.tile([128, 512], F32, name="z", tag="z", bufs=4)
                    nc.vector.tensor_copy(z[:], psZ[:])
                    # SWDGE on the otherwise-idle GPSIMD queue: keeps the SP
                    # queue free for the latency-sensitive XBAR transposes
                    nc.gpsimd.dma_start(
                        y.ap()[tc16 * 128:(tc16 + 1) * 128,
                               co2 * 512:(co2 + 1) * 512],
                        z[:],
                    )

            # ---------------- emission schedule ----------------
            for item in all_qkv_items(0):
                item()
            for tc16 in range(4):
                emit_v_chunk(tc16)

            for p in range(PAIRS):
                fillers = []
                if p == 0:
                    fillers += [lambda t=t: emit_v_chunk(t) for t in range(4, TC)]
                if p + 1 < PAIRS:
                    fillers += all_qkv_items(p + 1)
                for g in range(TG):
                    if p == PAIRS - 1 and g >= 2:
                        hi = 4 * (g - 1) if g < TG - 1 else 4 * g
                        fillers += [lambda t=t: emit_proj_chunk(t)
                                    for t in range(4 * (g - 2), hi)]
                    emit_attention_group(p, g, fillers,
                                         tail_proj=(p == PAIRS - 1 and
                                                    g == TG - 1))
                for f in fillers:
                    f()

    nc.compile()
    return nc


def _get_compiled():
    global _compiled
    if _compiled is None:
        _compiled = _build()
    return _compiled


F8NP = ml_dtypes.float8_e4m3


def _split_fp8(a):
    """a (f32) -> (a8, ar, a8s): a ~= a8 + ar exactly up to fp8 rounding of
    the residual; a8s = a8/64 pairs with 64x-scaled W residuals."""
    a8 = a.astype(F8NP)
    a8f = a8.astype(np.float32)
    ar = (a - a8f).astype(F8NP)
    a8s = (a8f / 64.0).astype(F8NP)
    return a8, ar, a8s


def kernel(x, W_attn, W_proj, _trace=False):
    x = np.asarray(x)
    W_attn = np.asarray(W_attn)
    W_proj = np.asarray(W_proj)
    nc = _get_compiled()

    tri = np.triu(np.ones((128, 128), np.float32)).astype(ml_dtypes.bfloat16)

    # per-batch x splits (shared by the two head-group cores)
    xsplits = []
    for b in range(B):
        xT = np.ascontiguousarray(x[b].T).astype(np.float32)
        xsplits.append(_split_fp8(xT))

    in_maps = []
    for core in range(N_CORES):
        b, hg = core // 2, core % 2
        cols = slice(hg * 512, (hg + 1) * 512)
        x8, xr, x8s = xsplits[b]
        m = {"x8": x8, "xr": xr, "x8s": x8s, "tri": tri,
             "wp": W_proj[hg * 512:(hg + 1) * 512, :].astype(ml_dtypes.bfloat16)}
        for nm, wfull in (("q", W_attn[:, 0 * C:1 * C]),
                          ("k", W_attn[:, 1 * C:2 * C]),
                          ("v", W_attn[:, 2 * C:3 * C])):
            w = wfull[:, cols].astype(np.float32)
            w8 = w.astype(F8NP)
            wr = ((w - w8.astype(np.float32)) * 64.0).astype(F8NP)
            m[f"w8{nm}"] = w8
            m[f"wr{nm}"] = wr
        in_maps.append(m)

    res = run_bass_kernel_spmd(nc, in_maps, list(range(N_CORES)), trace=_trace)
    out = np.empty((B, T, C), np.float32)
    for b in range(B):
        out[b] = res.results[2 * b]["y"] + res.results[2 * b + 1]["y"]
    if _trace:
        kernel._last_exec_time_ns = res.exec_time_ns
        kernel._last_results = res
    return out


# revision 17
# speedup vs baseline: 1.0259x; 1.0259x over previous
"""Causal self-attention kernel for 8 TRN2 NeuronCores.

Problem: B=4, T=2048, C=1024, H=16 heads, D=64 (fp32 in/out).

Sharding: 8 cores = 4 batch entries x 2 head-groups (8 heads each).
Each core computes, for its (batch b, head-group hg):
    qkv slice -> flash-style causal attention (no-max softmax) -> partial
    projection y_part = attn_out @ W_proj[rows of its heads].
Host sums the two partial projections per batch entry.

Key optimizations over the 262us baseline:
  - qkv projections run as fp8e4m3 DoubleRow matmuls (0.5 cyc/row, 2 k-tiles
    per instruction => 2.7x bf16 MAC throughput).  Accuracy is preserved with
    a 3-term compensated product:
        x@W ~= x8@w8 + xr@w8 + (x8/64)@(wr*64)
    where x8=fp8(x), xr=fp8(x-x8), w8=fp8(W), wr=W-w8.  Measured error is
    BETTER than bf16 (residuals capture the quantization error; only the
    xr@wr cross term ~0.07% is dropped).  All splits are precomputed on host.
  - att@V is restructured: stationary = P^T chunk [128k x 128q], moving =
    [v_h | ones] (65 cols), output psY[128 queries, 65] -- full 128 output
    partitions instead of 65, halving PE time vs the baseline layout.  Row 64
    of psY is the softmax denominator for free.
  - normalization is a per-partition broadcast multiply (recip of den column),
    no GPSIMD partition_broadcast needed.
  - the [q, d] -> [d, q] transpose for the projection runs on the DMA XBAR
    (dma_start_transpose), costing no PE/DVE time.
  - PSUM: one start/stop per psum BANK per accumulation lifetime (the sim
    zeroes/tracks groups at 2KB granularity); the 2x2x65 psY accumulator
    regions inside one bank rely on deferred first-touch zeroing.

Cost-model (TimelineSim) breakdown per core: PE ~176us busy, ACT ~146us
(exp), DVE ~77us (copies, tri mask, normalize), DMA ~60us.
"""

import numpy as np
import ml_dtypes
import sys

sys.path.insert(0, "/opt/trn_rl_repo")

import concourse.bass as bass
import concourse.mybir as mybir
import concourse.tile as tile
from concourse import bacc
from concourse.bass_utils import run_bass_kernel_spmd

BF = mybir.dt.bfloat16
F8 = mybir.dt.float8e4
F32 = mybir.dt.float32
AF = mybir.ActivationFunctionType
DR = mybir.MatmulPerfMode.DoubleRow

B, T, C = 4, 2048, 1024
H, D = 16, 64
N_CORES = 8
HEADS_PER_CORE = 8          # 4 pairs
PAIRS = 4
TC = T // 128               # 16 t-chunks of 128
TG = T // 512               # 4 t-groups of 512
CT = C // 128               # 8 contraction tiles of 128

_compiled = None


def _build():
    nc = bacc.Bacc("TRN2", target_bir_lowering=False)

    x8 = nc.declare_dram_parameter("x8", [C, T], F8, isOutput=False)
    xr = nc.declare_dram_parameter("xr", [C, T], F8, isOutput=False)
    x8s = nc.declare_dram_parameter("x8s", [C, T], F8, isOutput=False)
    w8q = nc.declare_dram_parameter("w8q", [C, 512], F8, isOutput=False)
    w8k = nc.declare_dram_parameter("w8k", [C, 512], F8, isOutput=False)
    w8v = nc.declare_dram_parameter("w8v", [C, 512], F8, isOutput=False)
    wrq = nc.declare_dram_parameter("wrq", [C, 512], F8, isOutput=False)
    wrk = nc.declare_dram_parameter("wrk", [C, 512], F8, isOutput=False)
    wrv = nc.declare_dram_parameter("wrv", [C, 512], F8, isOutput=False)
    wp = nc.declare_dram_parameter("wp", [512, C], BF, isOutput=False)
    tri = nc.declare_dram_parameter("tri", [128, 128], BF, isOutput=False)
    y = nc.declare_dram_parameter("y", [T, C], F32, isOutput=True)

    with tile.TileContext(nc) as tc:
        with (
            tc.tile_pool(name="const", bufs=1) as cpool,
            tc.tile_pool(name="small", bufs=2) as spool,
            tc.tile_pool(name="ps_s", bufs=2, space="PSUM") as ps_s,
            tc.tile_pool(name="ps_sm", bufs=4, space="PSUM") as ps_sm,
        ):
            # ---------------- SBUF tiles ----------------
            x8_t = cpool.tile([128, CT, T], F8, name="x8_t", tag="x8_t")
            xr_t = cpool.tile([128, CT, T], F8, name="xr_t", tag="xr_t")
            x8s_t = cpool.tile([128, CT, T], F8, name="x8s_t", tag="x8s_t")
            w8_t = {}
            wr_t = {}
            for nm, w8d, wrd in (("q", w8q, wrq), ("k", w8k, wrk), ("v", w8v, wrv)):
                w8_t[nm] = cpool.tile([128, CT, 512], F8, name=f"w8{nm}_t",
                                      tag=f"w8{nm}_t")
                wr_t[nm] = cpool.tile([128, CT, 512], F8, name=f"wr{nm}_t",
                                      tag=f"wr{nm}_t")
            wp_t = cpool.tile([128, PAIRS, C], BF, name="wp_t", tag="wp_t")
            tri_t = cpool.tile([128, 128], BF, name="tri_t", tag="tri_t")

            # v tiles: [t-chunk, head, 64 v dims | ones | pad]
            v_t = cpool.tile([128, TC, HEADS_PER_CORE, 66], BF, name="v_t",
                             tag="v_t")
            nc.vector.memset(v_t[:, :, :, 64:65], 1.0)

            q_t = [[cpool.tile([128, 512], BF, name=f"q_{p}_{g}", tag=f"q_{p}_{g}")
                    for g in range(TG)] for p in range(PAIRS)]
            k_t = [[cpool.tile([128, 512], BF, name=f"k_{p}_{g}", tag=f"k_{p}_{g}")
                    for g in range(TG)] for p in range(PAIRS)]
            # transposed attention output, one [128 = 2h x 64d, 128 t] tile per
            # (pair, group, qtile): the XBAR transpose needs an offset-0,
            # whole-tile destination (nonzero free offsets corrupt the output)
            yT2_t = {(p, g, qt): cpool.tile([128, 128], BF,
                                            name=f"yT2_{p}_{g}_{qt}",
                                            tag=f"yT2_{p}_{g}_{qt}")
                     for p in range(PAIRS) for g in range(TG) for qt in range(4)}

            # ---------------- DMA loads ----------------
            x8_r = x8.ap().rearrange("(ct p) t -> p ct t", p=128)
            xr_r = xr.ap().rearrange("(ct p) t -> p ct t", p=128)
            x8s_r = x8s.ap().rearrange("(ct p) t -> p ct t", p=128)
            w8_r = {nm: d.ap().rearrange("(ct p) d -> p ct d", p=128)
                    for nm, d in (("q", w8q), ("k", w8k), ("v", w8v))}
            wr_r = {nm: d.ap().rearrange("(ct p) d -> p ct d", p=128)
                    for nm, d in (("q", wrq), ("k", wrk), ("v", wrv))}

            def load_xg(g):
                tg = slice(g * 512, (g + 1) * 512)
                nc.sync.dma_start(x8_t[:, :, tg], x8_r[:, :, tg])
                nc.sync.dma_start(xr_t[:, :, tg], xr_r[:, :, tg])
                nc.sync.dma_start(x8s_t[:, :, tg], x8s_r[:, :, tg])

            # startup-critical order: the first matmul (k of pair0/group0,
            # term x8@w8k) needs only x8 g0 + w8k
            nc.sync.dma_start(x8_t[:, 0:4, 0:512], x8_r[:, 0:4, 0:512])
            nc.sync.dma_start(w8_t["k"][:, 0:4], w8_r["k"][:, 0:4])
            nc.sync.dma_start(x8_t[:, 4:8, 0:512], x8_r[:, 4:8, 0:512])
            nc.sync.dma_start(w8_t["k"][:, 4:8], w8_r["k"][:, 4:8])
            nc.sync.dma_start(w8_t["q"][:], w8_r["q"][:])
            nc.sync.dma_start(xr_t[:, :, 0:512], xr_r[:, :, 0:512])
            nc.sync.dma_start(wr_t["k"][:], wr_r["k"][:])
            nc.sync.dma_start(wr_t["q"][:], wr_r["q"][:])
            nc.sync.dma_start(x8s_t[:, :, 0:512], x8s_r[:, :, 0:512])
            nc.sync.dma_start(tri_t[:], tri.ap())
            nc.sync.dma_start(w8_t["v"][:], w8_r["v"][:])
            nc.sync.dma_start(wr_t["v"][:], wr_r["v"][:])
            for g in range(1, TG):
                load_xg(g)
            nc.sync.dma_start(wp_t[:], wp.ap().rearrange("(pr p) co -> p pr co",
                                                         p=128))

            # ---------------- qkv work items (3-term fp8 DoubleRow) ----------
            def emit_v_chunk(tc16):
                psV = ps_sm.tile([128, 512], F32, name=f"psV_{tc16}", tag="util",
                                 bufs=2)
                tsl = slice(tc16 * 128, (tc16 + 1) * 128)
                terms = [(x8_t, w8_t["v"]), (xr_t, w8_t["v"]), (x8s_t, wr_t["v"])]
                for ti, (xa, wb) in enumerate(terms):
                    for ci in range(4):
                        nc.tensor.matmul(
                            psV[:],
                            xa[:, 2 * ci:2 * ci + 2, tsl],
                            wb[:, 2 * ci:2 * ci + 2, :],
                            start=(ti == 0 and ci == 0),
                            stop=(ti == 2 and ci == 3),
                            perf_mode=DR,
                        )
                nc.vector.tensor_copy(v_t[:, tc16, :, 0:64], psV[:])

            def emit_qk_group(p, g, which):
                dest = q_t if which == "q" else k_t
                ps = ps_sm.tile([128, 512], F32, name=f"ps{which}_{p}_{g}",
                                tag="util", bufs=2)
                psl = slice(p * 128, (p + 1) * 128)
                gsl = slice(g * 512, (g + 1) * 512)
                terms = [(w8_t[which], x8_t), (w8_t[which], xr_t),
                         (wr_t[which], x8s_t)]
                for ti, (wa, xb) in enumerate(terms):
                    for ci in range(4):
                        nc.tensor.matmul(
                            ps[:],
                            wa[:, 2 * ci:2 * ci + 2, psl],
                            xb[:, 2 * ci:2 * ci + 2, gsl],
                            start=(ti == 0 and ci == 0),
                            stop=(ti == 2 and ci == 3),
                            perf_mode=DR,
                        )
                nc.vector.tensor_copy(dest[p][g][:], ps[:])

            def all_qkv_items(p):
                items = []
                for g in range(TG):
                    items.append(lambda p=p, g=g: emit_qk_group(p, g, "k"))
                    items.append(lambda p=p, g=g: emit_qk_group(p, g, "q"))
                return items

            # ---------------- attention for one (pair, group) ----------------
            def emit_attention_group(p, g, fillers, tail_proj=False):
                nchunks = 4 * g + 4
                # two accumulator banks: lo = qtiles 0,1 / hi = qtiles 2,3
                # layout [128 q, qt%2, h, 65]; col 64 = denominator
                psY = [ps_sm.tile([128, 2, 2, 65], F32, name=f"psY{half}_{p}_{g}",
                                  tag="psY", bufs=2) for half in range(2)]

                def finalize_qt(qt):
                    ps = psY[qt // 2]
                    rec = spool.tile([128, 2, 1], F32, name="rec", tag="rec",
                                     bufs=4)
                    nc.vector.reciprocal(rec[:], ps[:, qt % 2, :, 64:65])
                    ysb = spool.tile([128, 2, 64], BF, name="ysb", tag="ysb",
                                     bufs=4)
                    nc.vector.tensor_mul(
                        ysb[:], ps[:, qt % 2, :, 0:64],
                        rec[:].to_broadcast([128, 2, 64]),
                    )
                    # [q, (h d)] -> [(h d), q] on the DMA crossbar
                    nc.sync.dma_start_transpose(yT2_t[(p, g, qt)][:], ysb[:])

                started = [False, False]

                def y_mm(pT, c, qt, h):
                    half = qt // 2
                    st = not started[half]
                    started[half] = True
                    # one start/stop per psum BANK: start on the first emitted
                    # matmul into the tile, stop on the very last (the sim
                    # zeroes/tracks accumulation groups per 2KB region)
                    stop = qt % 2 == 1 and h == 1 and c == 4 * g + qt
                    nc.tensor.matmul(
                        psY[half][:, qt % 2, h, 0:65],
                        pT[:, h, qt * 128:(qt + 1) * 128],
                        v_t[:, c, 2 * p + h, 0:65],
                        start=st, stop=stop,
                    )

                for c in range(nchunks):
                    diag = c >= 4 * g
                    jofs = 128 * (c - 4 * g) if diag else 0
                    kg, kc = c // 4, c % 4
                    psS = ps_s.tile([128, 2, 512], F32, name=f"psS_{p}_{g}_{c}",
                                    tag="s")
                    for h in range(2):
                        nc.tensor.matmul(
                            psS[:, h, jofs:512],
                            k_t[p][kg][h * 64:(h + 1) * 64,
                                       kc * 128:(kc + 1) * 128],
                            q_t[p][g][h * 64:(h + 1) * 64, jofs:512],
                            start=True, stop=True,
                        )
                    pT = spool.tile([128, 2, 512], BF, name="pT", tag="pT",
                                    bufs=6)
                    nc.scalar.activation(pT[:, :, jofs:512], psS[:, :, jofs:512],
                                         AF.Exp, scale=0.125)
                    qt_min = c - 4 * g if diag else 0
                    # non-diagonal qtiles first (they don't need the tri mask)
                    for qt in range(qt_min + 1, 4):
                        for h in range(2):
                            y_mm(pT, c, qt, h)
                    if diag:
                        nc.vector.tensor_mul(
                            pT[:, :, jofs:jofs + 128],
                            pT[:, :, jofs:jofs + 128],
                            tri_t[:, None, :].to_broadcast([128, 2, 128]),
                        )
                    for h in range(2):
                        y_mm(pT, c, qt_min, h)
                    if diag:
                        finalize_qt(qt_min)
                        if tail_proj and qt_min >= 1:
                            # last group: its proj chunks chase the transposes
                            emit_proj_chunk(4 * g + qt_min - 1)
                    if fillers and c % 2 == 1:
                        fillers.pop(0)()
                if tail_proj:
                    emit_proj_chunk(4 * g + 3)

            # ---------------- projection chunk ----------------
            def emit_proj_chunk(tc16):
                g16, qt16 = tc16 // 4, tc16 % 4
                for co2 in range(2):
                    psZ = ps_sm.tile([128, 512], F32, name=f"psZ_{tc16}_{co2}",
                                     tag="util", bufs=2)
                    for p in range(PAIRS):
                        nc.tensor.matmul(
                            psZ[:],
                            yT2_t[(p, g16, qt16)][:],
                            wp_t[:, p, co2 * 512:(co2 + 1) * 512],
                            start=(p == 0), stop=(p == PAIRS - 1),
                        )
                    z = spool.tile([128, 512], F32, name="z", tag="z", bufs=8)
                    nc.vector.tensor_copy(z[:], psZ[:])
                    # SWDGE on the otherwise-idle GPSIMD queue: keeps the SP
                    # queue free for the latency-sensitive XBAR transposes
                    nc.gpsimd.dma_start(
                        y.ap()[tc16 * 128:(tc16 + 1) * 128,
                               co2 * 512:(co2 + 1) * 512],
                        z[:],
                    )

            # ---------------- emission schedule ----------------
            for item in all_qkv_items(0):
                item()
            for tc16 in range(4):
                emit_v_chunk(tc16)

            for p in range(PAIRS):
                fillers = []
                if p == 0:
                    fillers += [lambda t=t: emit_v_chunk(t) for t in range(4, TC)]
                if p + 1 < PAIRS:
                    fillers += all_qkv_items(p + 1)
                for g in range(TG):
                    if p == PAIRS - 1 and g >= 2:
                        hi = 4 * (g - 1) if g < TG - 1 else 4 * g
                        fillers += [lambda t=t: emit_proj_chunk(t)
                                    for t in range(4 * (g - 2), hi)]
                    emit_attention_group(p, g, fillers,
                                         tail_proj=(p == PAIRS - 1 and
                                                    g == TG - 1))
                for f in fillers:
                    f()

    nc.compile()
    return nc


def _get_compiled():
    global _compiled
    if _compiled is None:
        _compiled = _build()
    return _compiled


F8NP = ml_dtypes.float8_e4m3


def _split_fp8(a):
    """a (f32) -> (a8, ar, a8s): a ~= a8 + ar exactly up to fp8 rounding of
    the residual; a8s = a8/64 pairs with 64x-scaled W residuals."""
    a8 = a.astype(F8NP)
    a8f = a8.astype(np.float32)
    ar = (a - a8f).astype(F8NP)
    a8s = (a8f / 64.0).astype(F8NP)
    return a8, ar, a8s


def kernel(x, W_attn, W_proj, _trace=False):
    x = np.asarray(x)
    W_attn = np.asarray(W_attn)
    W_proj = np.asarray(W_proj)
    nc = _get_compiled()

    tri = np.triu(np.ones((128, 128), np.float32)).astype(ml_dtypes.bfloat16)

    # per-batch x splits (shared by the two head-group cores)
    xsplits = []
    for b in range(B):
        xT = np.ascontiguousarray(x[b].T).astype(np.float32)
        xsplits.append(_split_fp8(xT))

    in_maps = []
    for core in range(N_CORES):
        b, hg = core // 2, core % 2
        cols = slice(hg * 512, (hg + 1) * 512)
        x8, xr, x8s = xsplits[b]
        m = {"x8": x8, "xr": xr, "x8s": x8s, "tri": tri,
             "wp": W_proj[hg * 512:(hg + 1) * 512, :].astype(ml_dtypes.bfloat16)}
        for nm, wfull in (("q", W_attn[:, 0 * C:1 * C]),
                          ("k", W_attn[:, 1 * C:2 * C]),
                          ("v", W_attn[:, 2 * C:3 * C])):
            w = wfull[:, cols].astype(np.float32)
            w8 = w.astype(F8NP)
            wr = ((w - w8.astype(np.float32)) * 64.0).astype(F8NP)
            m[f"w8{nm}"] = w8
            m[f"wr{nm}"] = wr
        in_maps.append(m)

    res = run_bass_kernel_spmd(nc, in_maps, list(range(N_CORES)), trace=_trace)
    out = np.empty((B, T, C), np.float32)
    for b in range(B):
        out[b] = res.results[2 * b]["y"] + res.results[2 * b + 1]["y"]
    if _trace:
        kernel._last_exec_time_ns = res.exec_time_ns
        kernel._last_results = res
    return out


# revision 18
# speedup vs baseline: 1.0811x; 1.0537x over previous
"""Causal self-attention kernel for 8 TRN2 NeuronCores.

Problem: B=4, T=2048, C=1024, H=16 heads, D=64 (fp32 in/out).

Sharding: 8 cores = 4 batch entries x 2 head-groups (8 heads each).
Each core computes, for its (batch b, head-group hg):
    qkv slice -> flash-style causal attention (no-max softmax) -> partial
    projection y_part = attn_out @ W_proj[rows of its heads].
Host sums the two partial projections per batch entry.

Key optimizations over the 262us baseline:
  - qkv projections run as fp8e4m3 DoubleRow matmuls (0.5 cyc/row, 2 k-tiles
    per instruction => 2.7x bf16 MAC throughput).  Accuracy is preserved with
    a 3-term compensated product:
        x@W ~= x8@w8 + xr@w8 + (x8/64)@(wr*64)
    where x8=fp8(x), xr=fp8(x-x8), w8=fp8(W), wr=W-w8.  Measured error is
    BETTER than bf16 (residuals capture the quantization error; only the
    xr@wr cross term ~0.07% is dropped).  All splits are precomputed on host.
  - att@V is restructured: stationary = P^T chunk [128k x 128q], moving =
    [v_h | ones] (65 cols), output psY[128 queries, 65] -- full 128 output
    partitions instead of 65, halving PE time vs the baseline layout.  Row 64
    of psY is the softmax denominator for free.
  - normalization is a per-partition broadcast multiply (recip of den column),
    no GPSIMD partition_broadcast needed.
  - the [q, d] -> [d, q] transpose for the projection runs on the DMA XBAR
    (dma_start_transpose), costing no PE/DVE time.
  - PSUM: one start/stop per psum BANK per accumulation lifetime (the sim
    zeroes/tracks groups at 2KB granularity); the 2x2x65 psY accumulator
    regions inside one bank rely on deferred first-touch zeroing.

Cost-model (TimelineSim) breakdown per core: PE ~176us busy, ACT ~146us
(exp), DVE ~77us (copies, tri mask, normalize), DMA ~60us.
"""

import numpy as np
import ml_dtypes
import sys

sys.path.insert(0, "/opt/trn_rl_repo")

import concourse.bass as bass
import concourse.mybir as mybir
import concourse.tile as tile
from concourse import bacc
from concourse.bass_utils import run_bass_kernel_spmd

BF = mybir.dt.bfloat16
F8 = mybir.dt.float8e4
F32 = mybir.dt.float32
AF = mybir.ActivationFunctionType
DR = mybir.MatmulPerfMode.DoubleRow

B, T, C = 4, 2048, 1024
H, D = 16, 64
N_CORES = 8
HEADS_PER_CORE = 8          # 4 pairs
PAIRS = 4
TC = T // 128               # 16 t-chunks of 128
TG = T // 512               # 4 t-groups of 512
CT = C // 128               # 8 contraction tiles of 128

_compiled = None


def _build():
    nc = bacc.Bacc("TRN2", target_bir_lowering=False)

    x8 = nc.declare_dram_parameter("x8", [C, T], F8, isOutput=False)
    xr = nc.declare_dram_parameter("xr", [C, T], F8, isOutput=False)
    x8s = nc.declare_dram_parameter("x8s", [C, T], F8, isOutput=False)
    w8q = nc.declare_dram_parameter("w8q", [C, 512], F8, isOutput=False)
    w8k = nc.declare_dram_parameter("w8k", [C, 512], F8, isOutput=False)
    w8v = nc.declare_dram_parameter("w8v", [C, 512], F8, isOutput=False)
    wrq = nc.declare_dram_parameter("wrq", [C, 512], F8, isOutput=False)
    wrk = nc.declare_dram_parameter("wrk", [C, 512], F8, isOutput=False)
    wrv = nc.declare_dram_parameter("wrv", [C, 512], F8, isOutput=False)
    wp = nc.declare_dram_parameter("wp", [512, C], BF, isOutput=False)
    tri = nc.declare_dram_parameter("tri", [128, 128], BF, isOutput=False)
    y = nc.declare_dram_parameter("y", [T, C], F32, isOutput=True)

    with tile.TileContext(nc) as tc:
        with (
            tc.tile_pool(name="const", bufs=1) as cpool,
            tc.tile_pool(name="small", bufs=2) as spool,
            tc.tile_pool(name="ps_s", bufs=2, space="PSUM") as ps_s,
            tc.tile_pool(name="ps_sm", bufs=4, space="PSUM") as ps_sm,
        ):
            # ---------------- SBUF tiles ----------------
            x8_t = cpool.tile([128, CT, T], F8, name="x8_t", tag="x8_t")
            xr_t = cpool.tile([128, CT, T], F8, name="xr_t", tag="xr_t")
            x8s_t = cpool.tile([128, CT, T], F8, name="x8s_t", tag="x8s_t")
            w8_t = {}
            wr_t = {}
            for nm, w8d, wrd in (("q", w8q, wrq), ("k", w8k, wrk), ("v", w8v, wrv)):
                w8_t[nm] = cpool.tile([128, CT, 512], F8, name=f"w8{nm}_t",
                                      tag=f"w8{nm}_t")
                wr_t[nm] = cpool.tile([128, CT, 512], F8, name=f"wr{nm}_t",
                                      tag=f"wr{nm}_t")
            wp_t = cpool.tile([128, PAIRS, C], BF, name="wp_t", tag="wp_t")
            tri_t = cpool.tile([128, 128], BF, name="tri_t", tag="tri_t")

            # v tiles: [t-chunk, head, 64 v dims | ones | pad]
            v_t = cpool.tile([128, TC, HEADS_PER_CORE, 66], BF, name="v_t",
                             tag="v_t")
            nc.vector.memset(v_t[:, :, :, 64:65], 1.0)

            q_t = [[cpool.tile([128, 512], BF, name=f"q_{p}_{g}", tag=f"q_{p}_{g}")
                    for g in range(TG)] for p in range(PAIRS)]
            k_t = [[cpool.tile([128, 512], BF, name=f"k_{p}_{g}", tag=f"k_{p}_{g}")
                    for g in range(TG)] for p in range(PAIRS)]
            # transposed attention output, one [128 = 2h x 64d, 128 t] tile per
            # (pair, group, qtile): the XBAR transpose needs an offset-0,
            # whole-tile destination (nonzero free offsets corrupt the output)
            yT2_t = {(p, g, qt): cpool.tile([128, 128], BF,
                                            name=f"yT2_{p}_{g}_{qt}",
                                            tag=f"yT2_{p}_{g}_{qt}")
                     for p in range(PAIRS) for g in range(TG) for qt in range(4)}

            # ---------------- DMA loads ----------------
            x8_r = x8.ap().rearrange("(ct p) t -> p ct t", p=128)
            xr_r = xr.ap().rearrange("(ct p) t -> p ct t", p=128)
            x8s_r = x8s.ap().rearrange("(ct p) t -> p ct t", p=128)
            w8_r = {nm: d.ap().rearrange("(ct p) d -> p ct d", p=128)
                    for nm, d in (("q", w8q), ("k", w8k), ("v", w8v))}
            wr_r = {nm: d.ap().rearrange("(ct p) d -> p ct d", p=128)
                    for nm, d in (("q", wrq), ("k", wrk), ("v", wrv))}

            def load_xg(g):
                tg = slice(g * 512, (g + 1) * 512)
                nc.sync.dma_start(x8_t[:, :, tg], x8_r[:, :, tg])
                nc.sync.dma_start(xr_t[:, :, tg], xr_r[:, :, tg])
                nc.sync.dma_start(x8s_t[:, :, tg], x8s_r[:, :, tg])

            # startup-critical order: the first matmul (k of pair0/group0,
            # term x8@w8k) needs only x8 g0 + w8k
            nc.sync.dma_start(x8_t[:, 0:4, 0:512], x8_r[:, 0:4, 0:512])
            nc.sync.dma_start(w8_t["k"][:, 0:4], w8_r["k"][:, 0:4])
            nc.sync.dma_start(x8_t[:, 4:8, 0:512], x8_r[:, 4:8, 0:512])
            nc.sync.dma_start(w8_t["k"][:, 4:8], w8_r["k"][:, 4:8])
            nc.sync.dma_start(w8_t["q"][:], w8_r["q"][:])
            nc.sync.dma_start(xr_t[:, :, 0:512], xr_r[:, :, 0:512])
            nc.sync.dma_start(wr_t["k"][:], wr_r["k"][:])
            nc.sync.dma_start(wr_t["q"][:], wr_r["q"][:])
            nc.sync.dma_start(x8s_t[:, :, 0:512], x8s_r[:, :, 0:512])
            nc.sync.dma_start(tri_t[:], tri.ap())
            nc.sync.dma_start(w8_t["v"][:], w8_r["v"][:])
            nc.sync.dma_start(wr_t["v"][:], wr_r["v"][:])
            for g in range(1, TG):
                load_xg(g)
            nc.sync.dma_start(wp_t[:], wp.ap().rearrange("(pr p) co -> p pr co",
                                                         p=128))

            # ---------------- qkv work items (3-term fp8 DoubleRow) ----------
            def emit_v_chunk(tc16):
                psV = ps_sm.tile([128, 512], F32, name=f"psV_{tc16}", tag="util",
                                 bufs=2)
                tsl = slice(tc16 * 128, (tc16 + 1) * 128)
                terms = [(x8_t, w8_t["v"]), (xr_t, w8_t["v"]), (x8s_t, wr_t["v"])]
                for ti, (xa, wb) in enumerate(terms):
                    for ci in range(4):
                        nc.tensor.matmul(
                            psV[:],
                            xa[:, 2 * ci:2 * ci + 2, tsl],
                            wb[:, 2 * ci:2 * ci + 2, :],
                            start=(ti == 0 and ci == 0),
                            stop=(ti == 2 and ci == 3),
                            perf_mode=DR,
                        )
                nc.vector.tensor_copy(v_t[:, tc16, :, 0:64], psV[:])

            def emit_qk_group(p, g, which):
                dest = q_t if which == "q" else k_t
                ps = ps_sm.tile([128, 512], F32, name=f"ps{which}_{p}_{g}",
                                tag="util", bufs=2)
                psl = slice(p * 128, (p + 1) * 128)
                gsl = slice(g * 512, (g + 1) * 512)
                terms = [(w8_t[which], x8_t), (w8_t[which], xr_t),
                         (wr_t[which], x8s_t)]
                for ti, (wa, xb) in enumerate(terms):
                    for ci in range(4):
                        nc.tensor.matmul(
                            ps[:],
                            wa[:, 2 * ci:2 * ci + 2, psl],
                            xb[:, 2 * ci:2 * ci + 2, gsl],
                            start=(ti == 0 and ci == 0),
                            stop=(ti == 2 and ci == 3),
                            perf_mode=DR,
                        )
                nc.vector.tensor_copy(dest[p][g][:], ps[:])

            def all_qkv_items(p):
                items = []
                for g in range(TG):
                    items.append(lambda p=p, g=g: emit_qk_group(p, g, "k"))
                    items.append(lambda p=p, g=g: emit_qk_group(p, g, "q"))
                return items

            # ---------------- attention for one (pair, group) ----------------
            def emit_attention_group(p, g, fillers, tail_proj=False):
                nchunks = 4 * g + 4
                # two accumulator banks: lo = qtiles 0,1 / hi = qtiles 2,3
                # layout [128 q, qt%2, h, 65]; col 64 = denominator
                psY = [ps_sm.tile([128, 2, 2, 65], F32, name=f"psY{half}_{p}_{g}",
                                  tag="psY", bufs=2) for half in range(2)]

                def finalize_qt(qt):
                    ps = psY[qt // 2]
                    rec = spool.tile([128, 2, 1], F32, name="rec", tag="rec",
                                     bufs=4)
                    nc.vector.reciprocal(rec[:], ps[:, qt % 2, :, 64:65])
                    ysb = spool.tile([128, 2, 64], BF, name="ysb", tag="ysb",
                                     bufs=4)
                    nc.vector.tensor_mul(
                        ysb[:], ps[:, qt % 2, :, 0:64],
                        rec[:].to_broadcast([128, 2, 64]),
                    )
                    # [q, (h d)] -> [(h d), q] on the DMA crossbar
                    nc.sync.dma_start_transpose(yT2_t[(p, g, qt)][:], ysb[:])

                started = [False, False]

                def y_mm(pT, c, qt, h):
                    half = qt // 2
                    st = not started[half]
                    started[half] = True
                    # one start/stop per psum BANK: start on the first emitted
                    # matmul into the tile, stop on the very last (the sim
                    # zeroes/tracks accumulation groups per 2KB region)
                    stop = qt % 2 == 1 and h == 1 and c == 4 * g + qt
                    nc.tensor.matmul(
                        psY[half][:, qt % 2, h, 0:65],
                        pT[:, h, qt * 128:(qt + 1) * 128],
                        v_t[:, c, 2 * p + h, 0:65],
                        start=st, stop=stop,
                    )

                for c in range(nchunks):
                    diag = c >= 4 * g
                    jofs = 128 * (c - 4 * g) if diag else 0
                    kg, kc = c // 4, c % 4
                    psS = ps_s.tile([128, 2, 512], F32, name=f"psS_{p}_{g}_{c}",
                                    tag="s")
                    for h in range(2):
                        nc.tensor.matmul(
                            psS[:, h, jofs:512],
                            k_t[p][kg][h * 64:(h + 1) * 64,
                                       kc * 128:(kc + 1) * 128],
                            q_t[p][g][h * 64:(h + 1) * 64, jofs:512],
                            start=True, stop=True,
                        )
                    pT = spool.tile([128, 2, 512], BF, name="pT", tag="pT",
                                    bufs=6)
                    nc.scalar.activation(pT[:, :, jofs:512], psS[:, :, jofs:512],
                                         AF.Exp, scale=0.125)
                    qt_min = c - 4 * g if diag else 0
                    # non-diagonal qtiles first (they don't need the tri mask)
                    for qt in range(qt_min + 1, 4):
                        for h in range(2):
                            y_mm(pT, c, qt, h)
                    if diag:
                        nc.vector.tensor_mul(
                            pT[:, :, jofs:jofs + 128],
                            pT[:, :, jofs:jofs + 128],
                            tri_t[:, None, :].to_broadcast([128, 2, 128]),
                        )
                    for h in range(2):
                        y_mm(pT, c, qt_min, h)
                    if diag:
                        finalize_qt(qt_min)
                        if tail_proj and qt_min >= 1:
                            # last group: its proj chunks chase the transposes
                            emit_proj_chunk(4 * g + qt_min - 1)
                    if fillers and c % 2 == 1:
                        fillers.pop(0)()
                if tail_proj:
                    emit_proj_chunk(4 * g + 3)

            # ---------------- projection chunk ----------------
            def emit_proj_chunk(tc16):
                g16, qt16 = tc16 // 4, tc16 % 4
                for co2 in range(2):
                    psZ = ps_sm.tile([128, 512], F32, name=f"psZ_{tc16}_{co2}",
                                     tag="util", bufs=2)
                    for p in range(PAIRS):
                        nc.tensor.matmul(
                            psZ[:],
                            yT2_t[(p, g16, qt16)][:],
                            wp_t[:, p, co2 * 512:(co2 + 1) * 512],
                            start=(p == 0), stop=(p == PAIRS - 1),
                        )
                    z = spool.tile([128, 512], F32, name="z", tag="z", bufs=8)
                    nc.vector.tensor_copy(z[:], psZ[:])
                    nc.sync.dma_start(
                        y.ap()[tc16 * 128:(tc16 + 1) * 128,
                               co2 * 512:(co2 + 1) * 512],
                        z[:],
                    )

            # ---------------- emission schedule ----------------
            for item in all_qkv_items(0):
                item()
            for tc16 in range(4):
                emit_v_chunk(tc16)

            for p in range(PAIRS):
                fillers = []
                if p == 0:
                    fillers += [lambda t=t: emit_v_chunk(t) for t in range(4, TC)]
                if p + 1 < PAIRS:
                    fillers += all_qkv_items(p + 1)
                for g in range(TG):
                    if p == PAIRS - 1 and g >= 2:
                        hi = 4 * (g - 1) if g < TG - 1 else 4 * g
                        fillers += [lambda t=t: emit_proj_chunk(t)
                                    for t in range(4 * (g - 2), hi)]
                    emit_attention_group(p, g, fillers,
                                         tail_proj=(p == PAIRS - 1 and
                                                    g == TG - 1))
                for f in fillers:
                    f()

    nc.compile()
    return nc


def _get_compiled():
    global _compiled
    if _compiled is None:
        _compiled = _build()
    return _compiled


F8NP = ml_dtypes.float8_e4m3


def _split_fp8(a):
    """a (f32) -> (a8, ar, a8s): a ~= a8 + ar exactly up to fp8 rounding of
    the residual; a8s = a8/64 pairs with 64x-scaled W residuals."""
    a8 = a.astype(F8NP)
    a8f = a8.astype(np.float32)
    ar = (a - a8f).astype(F8NP)
    a8s = (a8f / 64.0).astype(F8NP)
    return a8, ar, a8s


def kernel(x, W_attn, W_proj, _trace=False):
    x = np.asarray(x)
    W_attn = np.asarray(W_attn)
    W_proj = np.asarray(W_proj)
    nc = _get_compiled()

    tri = np.triu(np.ones((128, 128), np.float32)).astype(ml_dtypes.bfloat16)

    # per-batch x splits (shared by the two head-group cores)
    xsplits = []
    for b in range(B):
        xT = np.ascontiguousarray(x[b].T).astype(np.float32)
        xsplits.append(_split_fp8(xT))

    in_maps = []
    for core in range(N_CORES):
        b, hg = core // 2, core % 2
        cols = slice(hg * 512, (hg + 1) * 512)
        x8, xr, x8s = xsplits[b]
        m = {"x8": x8, "xr": xr, "x8s": x8s, "tri": tri,
             "wp": W_proj[hg * 512:(hg + 1) * 512, :].astype(ml_dtypes.bfloat16)}
        for nm, wfull in (("q", W_attn[:, 0 * C:1 * C]),
                          ("k", W_attn[:, 1 * C:2 * C]),
                          ("v", W_attn[:, 2 * C:3 * C])):
            w = wfull[:, cols].astype(np.float32)
            w8 = w.astype(F8NP)
            wr = ((w - w8.astype(np.float32)) * 64.0).astype(F8NP)
            m[f"w8{nm}"] = w8
            m[f"wr{nm}"] = wr
        in_maps.append(m)

    res = run_bass_kernel_spmd(nc, in_maps, list(range(N_CORES)), trace=_trace)
    out = np.empty((B, T, C), np.float32)
    for b in range(B):
        out[b] = res.results[2 * b]["y"] + res.results[2 * b + 1]["y"]
    if _trace:
        kernel._last_exec_time_ns = res.exec_time_ns
        kernel._last_results = res
    return out
